# revision 7
# baseline (speedup 1.0000x reference)
"""Trainium2 Bass kernel for a transformer decoder block (self-attn + cross-attn + FFN,
each with residual AddNorm), distributed over 8 NeuronCores.

Sharding: core c -> (batch b = c//2, row-interleave h = c%2). Each core owns the
1024 query rows y[b, h::2] of one batch element. All phases (attention outputs,
layernorms, FFN) are row-local, so no collectives are needed. Interleaving the
causal rows (global q = 2*m + h) makes the causal skip pattern identical on
every core, so one SPMD program can statically skip fully-masked key tiles.

Layout strategy (avoids all on-chip transposes in attention):
  scores^T St[k, m] = K·Qᵀ via lhsT=Kᵀ (d-major), rhs=Qᵀ (d-major)
  softmax denominator via ones-vector matmul (sum over key partitions)
  attn_out[m, d] = expStᵀ·V via lhsT=expSt, rhs=V (natural row-major)
FFN: hᵀ[f, m] = relu(w1ᵀ·x2ᵀ + b1) via lhsT=w1 (natural), rhs=x2ᵀ;
     ff[m, d] = hᵀᵀ·w2 via lhsT=hᵀ, rhs=w2 (natural).
Only x1 and x2 need on-chip transposes (PE identity transpose, 64 tiles each).

Matmuls run in bf16 (fp32 accumulation in PSUM); softmax/layernorm math in fp32.
gamma3/beta3 are applied on the host after gathering (exact fp32); the inner
gammas/betas and biases are applied on-device.
"""
import functools

import numpy as np
import ml_dtypes

import concourse.bacc as bacc
import concourse.bass as bass
import concourse.mybir as mybir
import concourse.tile as tile
from concourse.bass_utils import run_bass_kernel_spmd
from concourse.masks import make_identity

BF16 = mybir.dt.bfloat16
F32 = mybir.dt.float32
AF = mybir.ActivationFunctionType
ALU = mybir.AluOpType

P = 128
B, S, D, DFF = 4, 2048, 1024, 4096
M = S // 2              # local query rows per core
LK = S                  # key length
NDC = D // P            # 8 contraction chunks over d
NKT = LK // P           # 16 key tiles
MBLK = 256              # query-block size in the attention phases
NMBLK = M // MBLK       # 4
FBLK = 256              # m-block size in the FFN phase
NFB = M // FBLK         # 4
NFT = DFF // P          # 32 f tiles
EPS = 1e-5
SCALE = 1.0 / np.sqrt(D).item()

bf = ml_dtypes.bfloat16


def _bcast_ap(handle, n):
    """DRAM [n] vector -> partition-broadcast AP [P, n] (stride-0 partition dim)."""
    ap = handle.ap()
    return bass.AP(ap.tensor, ap.offset, [[0, P]] + list(ap.ap))


def _layernorm(nc, small, raw, out, eps_t, gamma_t, beta_t):
    """out = (raw - mean)/sqrt(var+eps) * gamma + beta, rows on partitions."""
    stats = small.tile([P, 2, 6], F32, tag="stats", name="stats")
    nc.vector.bn_stats(stats[:, 0, :], raw[:, 0:512])
    nc.vector.bn_stats(stats[:, 1, :], raw[:, 512:1024])
    mv = small.tile([P, 2], F32, tag="mv", name="mv")
    nc.vector.bn_aggr(mv, stats)
    rstd = small.tile([P, 1], F32, tag="rstd", name="rstd")
    nc.scalar.activation(rstd, mv[:, 1:2], AF.Sqrt, bias=eps_t)
    nc.vector.reciprocal(rstd, rstd)
    nc.vector.tensor_scalar(out, raw, mv[:, 0:1], rstd, ALU.subtract, ALU.mult)
    if gamma_t is not None:
        nc.vector.tensor_mul(out, out, gamma_t)
    if beta_t is not None:
        nc.vector.tensor_add(out, out, beta_t)


import os
_NMBLK_LIM = int(os.environ.get("K_NMBLK", str(NMBLK)))


@functools.lru_cache(maxsize=2)
def build_nc(reps: int = 1, phases: int = 3):
    nc = bacc.Bacc("TRN2", target_bir_lowering=False, debug=False)

    # ---- I/O ----
    qT_d = nc.dram_tensor("qT", [D, M], BF16, kind="ExternalInput")
    kT_d = nc.dram_tensor("kT", [D, LK], BF16, kind="ExternalInput")
    v1_d = nc.dram_tensor("v1", [LK, D], BF16, kind="ExternalInput")
    zT_d = nc.dram_tensor("zT", [D, LK], BF16, kind="ExternalInput")
    v2_d = nc.dram_tensor("v2", [LK, D], BF16, kind="ExternalInput")
    yres_d = nc.dram_tensor("yres", [M, D], F32, kind="ExternalInput")
    mask_d = nc.dram_tensor("mask", [P, NKT, MBLK], BF16, kind="ExternalInput")
    w1_d = nc.dram_tensor("w1", [D, DFF], BF16, kind="ExternalInput")
    w2_d = nc.dram_tensor("w2", [DFF, D], BF16, kind="ExternalInput")
    b1c_d = nc.dram_tensor("b1c", [P, NFT], F32, kind="ExternalInput")
    b2_d = nc.dram_tensor("b2v", [D], F32, kind="ExternalInput")
    g1_d = nc.dram_tensor("g1v", [D], BF16, kind="ExternalInput")
    be1_d = nc.dram_tensor("be1v", [D], BF16, kind="ExternalInput")
    g2_d = nc.dram_tensor("g2v", [D], BF16, kind="ExternalInput")
    be2_d = nc.dram_tensor("be2v", [D], BF16, kind="ExternalInput")
    out_d = nc.dram_tensor("out", [M, D], F32, kind="ExternalOutput")

    x1_d = nc.dram_tensor("x1_scratch", [M, D], F32)
    x2_d = nc.dram_tensor("x2_scratch", [M, D], F32)

    with tile.TileContext(nc) as tc:
        with (
            tc.tile_pool(name="const", bufs=1) as const,
            tc.tile_pool(name="persist", bufs=1) as persist,
            tc.tile_pool(name="est_p", bufs=3) as est_p,
            tc.tile_pool(name="resid_p", bufs=2) as resid_p,
            tc.tile_pool(name="raw_p", bufs=4) as raw_p,
            tc.tile_pool(name="lnout_p", bufs=2) as lnout_p,
            tc.tile_pool(name="w1_p", bufs=2) as w1_p,
            tc.tile_pool(name="w2_p", bufs=3) as w2_p,
            tc.tile_pool(name="small", bufs=4) as small,
            tc.tile_pool(name="dramsc", bufs=2, space="DRAM") as dramsc,
            tc.tile_pool(name="psum", bufs=1, space="PSUM") as psum,
        ):
            # ---- constants ----
            ones_t = const.tile([P, 1], BF16, name="ones_t")
            nc.vector.memset(ones_t, 1.0)
            eps_t = const.tile([P, 1], F32, name="eps_t")
            nc.vector.memset(eps_t, EPS)
            ident = const.tile([P, P], F32, name="ident")
            make_identity(nc, ident)
            b1c_t = const.tile([P, NFT], F32, name="b1c_t")
            nc.sync.dma_start(b1c_t, b1c_d.ap())
            b2_t = const.tile([P, D], F32, name="b2_t")
            nc.sync.dma_start(b2_t, _bcast_ap(b2_d, D))
            g1_t = const.tile([P, D], BF16, name="g1_t")
            nc.sync.dma_start(g1_t, _bcast_ap(g1_d, D))
            be1_t = const.tile([P, D], BF16, name="be1_t")
            nc.sync.dma_start(be1_t, _bcast_ap(be1_d, D))
            g2_t = const.tile([P, D], BF16, name="g2_t")
            nc.sync.dma_start(g2_t, _bcast_ap(g2_d, D))
            be2_t = const.tile([P, D], BF16, name="be2_t")
            nc.sync.dma_start(be2_t, _bcast_ap(be2_d, D))
            mask_t = const.tile([P, NKT, MBLK], BF16, name="mask_t")
            nc.sync.dma_start(mask_t, mask_d.ap())

            def body(rep):
                # persistent per-phase tensors (tags shared across phases where
                # the successor is DMA-loaded, so reuse only gates a DMA)
                qT_sb = persist.tile([P, NDC, M], BF16, tag="qT", name=f"qT_sb{rep}")
                nc.sync.dma_start(qT_sb, qT_d.ap().rearrange("(c p) m -> p c m", p=P))
                x1T_sb = persist.tile([P, NDC, M], BF16, tag="x1T", name=f"x1T{rep}")
                x2T_sb = persist.tile([P, NDC, M], BF16, tag="x2T", name=f"x2T{rep}")

                def attention(kvT_d_, v_d_, q_sb, causal, resid_dram, spill_dram,
                              gamma_t, beta_t, xT_out, tagp):
                    kvT = persist.tile([P, NDC, LK], BF16, tag="kvT",
                                       name=f"kvT_{tagp}{rep}")
                    nc.sync.dma_start(
                        kvT, kvT_d_.ap().rearrange("(c p) k -> p c k", p=P))
                    v_sb = persist.tile([P, NKT, D], BF16, tag="V",
                                        name=f"v_{tagp}{rep}")
                    nc.sync.dma_start(
                        v_sb, v_d_.ap().rearrange("(c p) d -> p c d", p=P))

                    for mblk in range(_NMBLK_LIM):
                        nkt = 4 * mblk + 4 if causal else NKT
                        o_ps = [[psum.tile([P, 512], F32, tag="acc", bufs=4,
                                           name=f"ops{tagp}{rep}_{mblk}_{t}_{d_}")
                                 for d_ in range(2)] for t in range(2)]
                        cs_ps = psum.tile([1, MBLK], F32, tag="csum", bufs=1,
                                          name=f"cs{tagp}{rep}_{mblk}")
                        for kt in range(nkt):
                            st_ps = psum.tile([P, 512], F32, tag="st", bufs=3,
                                              name=f"st{tagp}{rep}_{mblk}_{kt}")
                            stv = st_ps[:, :MBLK]
                            for dc in range(NDC):
                                nc.tensor.matmul(
                                    stv,
                                    kvT[:, dc, kt * P:(kt + 1) * P],
                                    q_sb[:, dc, mblk * MBLK:(mblk + 1) * MBLK],
                                    start=(dc == 0), stop=(dc == NDC - 1))
                            est = est_p.tile([P, MBLK], BF16, tag="est",
                                             name=f"est{tagp}{rep}_{mblk}_{kt}")
                            nc.scalar.activation(est, stv, AF.Exp, scale=SCALE)
                            if causal and kt >= 4 * mblk:
                                nc.vector.tensor_mul(est, est, mask_t[:, kt, :])
                            nc.tensor.matmul(cs_ps, ones_t, est,
                                             start=(kt == 0), stop=(kt == nkt - 1))
                            for mt2 in range(2):
                                for d_ in range(2):
                                    nc.tensor.matmul(
                                        o_ps[mt2][d_],
                                        est[:, mt2 * P:(mt2 + 1) * P],
                                        v_sb[:, kt, d_ * 512:(d_ + 1) * 512],
                                        start=(kt == 0), stop=(kt == nkt - 1))
                        # denominators: psum row -> dram bounce -> [P, 2] -> 1/x
                        cs_sb = small.tile([1, MBLK], F32, tag="cs_sb",
                                           name=f"cssb{tagp}{rep}_{mblk}")
                        nc.vector.tensor_copy(cs_sb, cs_ps)
                        cs_dr = dramsc.tile([1, MBLK], F32, tag="cs_dr",
                                            name=f"csdr{tagp}{rep}_{mblk}")
                        nc.sync.dma_start(cs_dr, cs_sb)
                        rec = small.tile([P, 2], F32, tag="rec",
                                         name=f"rec{tagp}{rep}_{mblk}")
                        nc.sync.dma_start(
                            rec, cs_dr.rearrange("o (t p) -> (o p) t", p=P))
                        nc.vector.reciprocal(rec, rec)

                        for mt2 in range(2):
                            mt = 2 * mblk + mt2
                            res_t = resid_p.tile([P, D], F32, tag="res",
                                                 name=f"res{tagp}{rep}_{mt}")
                            nc.sync.dma_start(
                                res_t, resid_dram.ap()[mt * P:(mt + 1) * P, :])
                            raw = raw_p.tile([P, D], F32, tag="raw",
                                             name=f"raw{tagp}{rep}_{mt}")
                            for d_ in range(2):
                                nc.vector.scalar_tensor_tensor(
                                    raw[:, d_ * 512:(d_ + 1) * 512],
                                    o_ps[mt2][d_], rec[:, mt2:mt2 + 1],
                                    res_t[:, d_ * 512:(d_ + 1) * 512],
                                    ALU.mult, ALU.add)
                            xo = lnout_p.tile([P, D], F32, tag="lnout",
                                              name=f"xo{tagp}{rep}_{mt}")
                            _layernorm(nc, small, raw, xo, eps_t, gamma_t, beta_t)
                            nc.sync.dma_start(
                                spill_dram.ap()[mt * P:(mt + 1) * P, :], xo)
                            for dcol in range(NDC):
                                tp = psum.tile([P, 512], F32, tag="st", bufs=3,
                                               name=f"tp{tagp}{rep}_{mt}_{dcol}")
                                nc.tensor.transpose(
                                    tp[:, :P], xo[:, dcol * P:(dcol + 1) * P], ident)
                                nc.vector.tensor_copy(
                                    xT_out[:, dcol, mt * P:(mt + 1) * P],
                                    tp[:, :P])

                attention(kT_d, v1_d, qT_sb, True, yres_d,
                          x1_d if phases >= 2 else out_d,
                          g1_t, be1_t, x1T_sb, "s")
                if phases < 2:
                    return
                attention(zT_d, v2_d, x1T_sb, False, x1_d,
                          x2_d if phases >= 3 else out_d,
                          g2_t, be2_t, x2T_sb, "c")
                if phases < 3:
                    return

                # ---- FFN + final AddNorm (gamma3/beta3 applied on host) ----
                hT = persist.tile([P, NFT, FBLK], BF16, tag="hT", name=f"hT{rep}")
                for mb in range(NFB):
                    for fg in range(NFT // 2):
                        w1c = w1_p.tile([P, NDC, 2 * P], BF16, tag="w1c",
                                        name=f"w1c{rep}_{mb}_{fg}")
                        nc.sync.dma_start(
                            w1c,
                            w1_d.ap().rearrange("(c p) f -> p c f", p=P)
                            [:, :, fg * 2 * P:(fg + 1) * 2 * P])
                        for f2 in range(2):
                            ft = fg * 2 + f2
                            h_ps = psum.tile([P, 512], F32, tag="st", bufs=3,
                                             name=f"hps{rep}_{mb}_{ft}")
                            hv = h_ps[:, :FBLK]
                            for dc in range(NDC):
                                nc.tensor.matmul(
                                    hv,
                                    w1c[:, dc, f2 * P:(f2 + 1) * P],
                                    x2T_sb[:, dc, mb * FBLK:(mb + 1) * FBLK],
                                    start=(dc == 0), stop=(dc == NDC - 1))
                            nc.scalar.activation(hT[:, ft, :], hv, AF.Relu,
                                                 bias=b1c_t[:, ft:ft + 1])
                    ff_ps = [[psum.tile([P, 512], F32, tag="acc", bufs=4,
                                        name=f"ffps{rep}_{mb}_{t}_{d_}")
                              for d_ in range(2)] for t in range(2)]
                    for d_ in range(2):
                        for ft in range(NFT):
                            w2c = w2_p.tile([P, 512], BF16, tag="w2c",
                                            name=f"w2c{rep}_{mb}_{d_}_{ft}")
                            nc.sync.dma_start(
                                w2c,
                                w2_d.ap()[ft * P:(ft + 1) * P,
                                          d_ * 512:(d_ + 1) * 512])
                            for mt2 in range(2):
                                nc.tensor.matmul(
                                    ff_ps[mt2][d_],
                                    hT[:, ft, mt2 * P:(mt2 + 1) * P],
                                    w2c,
                                    start=(ft == 0), stop=(ft == NFT - 1))
                    for mt2 in range(2):
                        mt = 2 * mb + mt2
                        res_t = resid_p.tile([P, D], F32, tag="res",
                                             name=f"resf{rep}_{mt}")
                        nc.sync.dma_start(
                            res_t, x2_d.ap()[mt * P:(mt + 1) * P, :])
                        raw = raw_p.tile([P, D], F32, tag="raw",
                                         name=f"rawf{rep}_{mt}")
                        for d_ in range(2):
                            sl = slice(d_ * 512, (d_ + 1) * 512)
                            nc.vector.tensor_add(raw[:, sl], ff_ps[mt2][d_],
                                                 res_t[:, sl])
                            nc.vector.tensor_add(raw[:, sl], raw[:, sl],
                                                 b2_t[:, sl])
                        xo = lnout_p.tile([P, D], F32, tag="lnout",
                                          name=f"xof{rep}_{mt}")
                        _layernorm(nc, small, raw, xo, eps_t, None, None)
                        nc.sync.dma_start(out_d.ap()[mt * P:(mt + 1) * P, :], xo)

            if reps == 1:
                body(0)
            else:
                for r in range(reps):
                    body(r)

    nc.compile()
    return nc


def _prep_core_inputs(y, Z, w1b, w2b, b1c, b2, g1b, be1b, g2b, be2b, b_idx, h):
    yb = y[b_idx]
    zb = Z[b_idx]
    yb_bf = yb.astype(bf)
    zb_bf = zb.astype(bf)
    kT = np.ascontiguousarray(yb_bf.T)
    qT = np.ascontiguousarray(kT[:, h::2])
    zT = np.ascontiguousarray(zb_bf.T)
    yres = np.ascontiguousarray(yb[h::2])

    # mask[p, kt, j]: key (kt*128+p) visible to query (2*(256*(kt//4)+j)+h)
    p_i = np.arange(P)[:, None, None]
    kt_i = np.arange(NKT)[None, :, None]
    j_i = np.arange(MBLK)[None, None, :]
    k_idx = kt_i * P + p_i
    q_idx = 2 * (MBLK * (kt_i // 4) + j_i) + h
    mask = (k_idx <= q_idx).astype(bf)

    return {
        "qT": qT, "kT": kT, "v1": yb_bf, "zT": zT, "v2": zb_bf,
        "yres": yres, "mask": np.ascontiguousarray(mask),
        "w1": w1b, "w2": w2b, "b1c": b1c, "b2v": b2,
        "g1v": g1b, "be1v": be1b, "g2v": g2b, "be2v": be2b,
    }


def make_in_maps(y, Z, w1, b1, w2, b2, g1, beta1, g2, beta2):
    w1b = w1.astype(bf)
    w2b = w2.astype(bf)
    b1c = np.ascontiguousarray(b1.reshape(NFT, P).T.astype(np.float32))
    args = (y, Z, w1b, w2b, b1c, b2.astype(np.float32),
            g1.astype(bf), beta1.astype(bf), g2.astype(bf), beta2.astype(bf))
    return [_prep_core_inputs(*args, c // 2, c % 2) for c in range(8)]


def kernel(y, Z, w1, b1, w2, b2, g1, beta1, g2, beta2, g3, beta3):
    y = np.asarray(y, dtype=np.float32)
    Z = np.asarray(Z, dtype=np.float32)
    (w1, b1, w2, b2, g1, beta1, g2, beta2, g3, beta3) = [
        np.asarray(a, dtype=np.float32)
        for a in (w1, b1, w2, b2, g1, beta1, g2, beta2, g3, beta3)]

    in_maps = make_in_maps(y, Z, w1, b1, w2, b2, g1, beta1, g2, beta2)
    nc = build_nc(1)
    res = run_bass_kernel_spmd(nc, in_maps, core_ids=list(range(8)), trace=False)

    out = np.empty((B, S, D), np.float32)
    for c in range(8):
        out[c // 2, c % 2::2, :] = res.results[c]["out"]
    # final gamma/beta exact in fp32 on host
    if not (np.all(g3 == 1.0) and np.all(beta3 == 0.0)):
        out = out * g3 + beta3
    return out


# revision 8
# speedup vs baseline: 5493.3464x; 5493.3464x over previous
"""Trainium2 Bass kernel for a transformer decoder block (self-attn + cross-attn + FFN,
each with residual AddNorm), distributed over 8 NeuronCores.

Sharding: core c -> (batch b = c//2, row-interleave h = c%2). Each core owns the
1024 query rows y[b, h::2] of one batch element. All phases (attention outputs,
layernorms, FFN) are row-local, so no collectives are needed. Interleaving the
causal rows (global q = 2*m + h) makes the causal skip pattern identical on
every core, so one SPMD program can statically skip fully-masked key tiles.

Layout strategy (avoids all on-chip transposes in attention):
  scores^T St[k, m] = K·Qᵀ via lhsT=Kᵀ (d-major), rhs=Qᵀ (d-major)
  softmax denominator via ones-vector matmul (sum over key partitions)
  attn_out[m, d] = expStᵀ·V via lhsT=expSt, rhs=V (natural row-major)
FFN: hᵀ[f, m] = relu(w1ᵀ·x2ᵀ + b1) via lhsT=w1 (natural), rhs=x2ᵀ;
     ff[m, d] = hᵀᵀ·w2 via lhsT=hᵀ, rhs=w2 (natural).
Only x1 and x2 need on-chip transposes (PE identity transpose, 64 tiles each).

Matmuls run in bf16 (fp32 accumulation in PSUM); softmax/layernorm math in fp32.
gamma3/beta3 are applied on the host after gathering (exact fp32); the inner
gammas/betas and biases are applied on-device.
"""
import functools

import numpy as np
import ml_dtypes

import concourse.bacc as bacc
import concourse.bass as bass
import concourse.mybir as mybir
import concourse.tile as tile
from concourse.bass_utils import run_bass_kernel_spmd
from concourse.masks import make_identity

BF16 = mybir.dt.bfloat16
F32 = mybir.dt.float32
AF = mybir.ActivationFunctionType
ALU = mybir.AluOpType

P = 128
B, S, D, DFF = 4, 2048, 1024, 4096
M = S // 2              # local query rows per core
LK = S                  # key length
NDC = D // P            # 8 contraction chunks over d
NKT = LK // P           # 16 key tiles
MBLK = 256              # query-block size in the attention phases
NMBLK = M // MBLK       # 4
FBLK = 256              # m-block size in the FFN phase
NFB = M // FBLK         # 4
NFT = DFF // P          # 32 f tiles
EPS = 1e-5
SCALE = 1.0 / np.sqrt(D).item()

bf = ml_dtypes.bfloat16


def _bcast_ap(handle, n):
    """DRAM [n] vector -> partition-broadcast AP [P, n] (stride-0 partition dim)."""
    ap = handle.ap()
    return bass.AP(ap.tensor, ap.offset, [[0, P]] + list(ap.ap))


def _layernorm(nc, small, raw, out, eps_t, gamma_t, beta_t):
    """out = (raw - mean)/sqrt(var+eps) * gamma + beta, rows on partitions."""
    stats = small.tile([P, 2, 6], F32, tag="stats", name="stats")
    nc.vector.bn_stats(stats[:, 0, :], raw[:, 0:512])
    nc.vector.bn_stats(stats[:, 1, :], raw[:, 512:1024])
    mv = small.tile([P, 2], F32, tag="mv", name="mv")
    nc.vector.bn_aggr(mv, stats)
    rstd = small.tile([P, 1], F32, tag="rstd", name="rstd")
    nc.scalar.activation(rstd, mv[:, 1:2], AF.Sqrt, bias=eps_t)
    nc.vector.reciprocal(rstd, rstd)
    nc.vector.tensor_scalar(out, raw, mv[:, 0:1], rstd, ALU.subtract, ALU.mult)
    if gamma_t is not None:
        nc.vector.tensor_mul(out, out, gamma_t)
    if beta_t is not None:
        nc.vector.tensor_add(out, out, beta_t)


import os
_NMBLK_LIM = int(os.environ.get("K_NMBLK", str(NMBLK)))


@functools.lru_cache(maxsize=2)
def build_nc(reps: int = 1, phases: int = 3):
    nc = bacc.Bacc("TRN2", target_bir_lowering=False, debug=False)

    # ---- I/O ----
    qT_d = nc.dram_tensor("qT", [D, M], BF16, kind="ExternalInput")
    kT_d = nc.dram_tensor("kT", [D, LK], BF16, kind="ExternalInput")
    v1_d = nc.dram_tensor("v1", [LK, D], BF16, kind="ExternalInput")
    zT_d = nc.dram_tensor("zT", [D, LK], BF16, kind="ExternalInput")
    v2_d = nc.dram_tensor("v2", [LK, D], BF16, kind="ExternalInput")
    yres_d = nc.dram_tensor("yres", [M, D], F32, kind="ExternalInput")
    mask_d = nc.dram_tensor("mask", [P, NKT, MBLK], BF16, kind="ExternalInput")
    w1_d = nc.dram_tensor("w1", [D, DFF], BF16, kind="ExternalInput")
    w2_d = nc.dram_tensor("w2", [DFF, D], BF16, kind="ExternalInput")
    b1c_d = nc.dram_tensor("b1c", [P, NFT], F32, kind="ExternalInput")
    b2_d = nc.dram_tensor("b2v", [D], F32, kind="ExternalInput")
    g1_d = nc.dram_tensor("g1v", [D], BF16, kind="ExternalInput")
    be1_d = nc.dram_tensor("be1v", [D], BF16, kind="ExternalInput")
    g2_d = nc.dram_tensor("g2v", [D], BF16, kind="ExternalInput")
    be2_d = nc.dram_tensor("be2v", [D], BF16, kind="ExternalInput")
    out_d = nc.dram_tensor("out", [M, D], F32, kind="ExternalOutput")

    x1_d = nc.dram_tensor("x1_scratch", [M, D], F32)
    x2_d = nc.dram_tensor("x2_scratch", [M, D], F32)

    with tile.TileContext(nc) as tc:
        with (
            tc.tile_pool(name="const", bufs=1) as const,
            tc.tile_pool(name="persist", bufs=1) as persist,
            tc.tile_pool(name="est_p", bufs=3) as est_p,
            tc.tile_pool(name="resid_p", bufs=2) as resid_p,
            tc.tile_pool(name="raw_p", bufs=4) as raw_p,
            tc.tile_pool(name="lnout_p", bufs=2) as lnout_p,
            tc.tile_pool(name="w1_p", bufs=2) as w1_p,
            tc.tile_pool(name="w2_p", bufs=3) as w2_p,
            tc.tile_pool(name="small", bufs=4) as small,
            tc.tile_pool(name="dramsc", bufs=2, space="DRAM") as dramsc,
            tc.tile_pool(name="psum", bufs=1, space="PSUM") as psum,
        ):
            # ---- constants ----
            ones_t = const.tile([P, 1], BF16, name="ones_t")
            nc.vector.memset(ones_t, 1.0)
            eps_t = const.tile([P, 1], F32, name="eps_t")
            nc.vector.memset(eps_t, EPS)
            ident = const.tile([P, P], F32, name="ident")
            make_identity(nc, ident)
            b1c_t = const.tile([P, NFT], F32, name="b1c_t")
            nc.sync.dma_start(b1c_t, b1c_d.ap())
            b2_t = const.tile([P, D], F32, name="b2_t")
            nc.sync.dma_start(b2_t, _bcast_ap(b2_d, D))
            g1_t = const.tile([P, D], BF16, name="g1_t")
            nc.sync.dma_start(g1_t, _bcast_ap(g1_d, D))
            be1_t = const.tile([P, D], BF16, name="be1_t")
            nc.sync.dma_start(be1_t, _bcast_ap(be1_d, D))
            g2_t = const.tile([P, D], BF16, name="g2_t")
            nc.sync.dma_start(g2_t, _bcast_ap(g2_d, D))
            be2_t = const.tile([P, D], BF16, name="be2_t")
            nc.sync.dma_start(be2_t, _bcast_ap(be2_d, D))
            mask_t = const.tile([P, NKT, MBLK], BF16, name="mask_t")
            nc.sync.dma_start(mask_t, mask_d.ap())

            def body(rep):
                # persistent per-phase tensors (tags shared across phases where
                # the successor is DMA-loaded, so reuse only gates a DMA)
                qT_sb = persist.tile([P, NDC, M], BF16, tag="qT", name=f"qT_sb{rep}")
                nc.sync.dma_start(qT_sb, qT_d.ap().rearrange("(c p) m -> p c m", p=P))
                x1T_sb = persist.tile([P, NDC, M], BF16, tag="x1T", name=f"x1T{rep}")
                x2T_sb = persist.tile([P, NDC, M], BF16, tag="x2T", name=f"x2T{rep}")

                def attention(kvT_d_, v_d_, q_sb, causal, resid_dram, spill_dram,
                              gamma_t, beta_t, xT_out, tagp):
                    kvT = persist.tile([P, NDC, LK], BF16, tag="kvT",
                                       name=f"kvT_{tagp}{rep}")
                    nc.sync.dma_start(
                        kvT, kvT_d_.ap().rearrange("(c p) k -> p c k", p=P))
                    v_sb = persist.tile([P, NKT, D], BF16, tag="V",
                                        name=f"v_{tagp}{rep}")
                    nc.sync.dma_start(
                        v_sb, v_d_.ap().rearrange("(c p) d -> p c d", p=P))

                    for mblk in range(_NMBLK_LIM):
                        nkt = 4 * mblk + 4 if causal else NKT
                        o_ps = [[psum.tile([P, 512], F32, tag="acc", bufs=4,
                                           name=f"ops{tagp}{rep}_{mblk}_{t}_{d_}")
                                 for d_ in range(2)] for t in range(2)]
                        cs_ps = psum.tile([1, MBLK], F32, tag="csum", bufs=1,
                                          name=f"cs{tagp}{rep}_{mblk}")
                        for kt in range(nkt):
                            st_ps = psum.tile([P, 512], F32, tag="st", bufs=3,
                                              name=f"st{tagp}{rep}_{mblk}_{kt}")
                            stv = st_ps[:, :MBLK]
                            for dc in range(NDC):
                                nc.tensor.matmul(
                                    stv,
                                    kvT[:, dc, kt * P:(kt + 1) * P],
                                    q_sb[:, dc, mblk * MBLK:(mblk + 1) * MBLK],
                                    start=(dc == 0), stop=(dc == NDC - 1))
                            est = est_p.tile([P, MBLK], BF16, tag="est",
                                             name=f"est{tagp}{rep}_{mblk}_{kt}")
                            nc.scalar.activation(est, stv, AF.Exp, scale=SCALE)
                            if causal and kt >= 4 * mblk:
                                nc.vector.tensor_mul(est, est, mask_t[:, kt, :])
                            nc.tensor.matmul(cs_ps, ones_t, est,
                                             start=(kt == 0), stop=(kt == nkt - 1))
                            for mt2 in range(2):
                                for d_ in range(2):
                                    nc.tensor.matmul(
                                        o_ps[mt2][d_],
                                        est[:, mt2 * P:(mt2 + 1) * P],
                                        v_sb[:, kt, d_ * 512:(d_ + 1) * 512],
                                        start=(kt == 0), stop=(kt == nkt - 1))
                        # denominators: psum row -> dram bounce -> [P, 2] -> 1/x
                        cs_sb = small.tile([1, MBLK], F32, tag="cs_sb",
                                           name=f"cssb{tagp}{rep}_{mblk}")
                        nc.vector.tensor_copy(cs_sb, cs_ps)
                        cs_dr = dramsc.tile([1, MBLK], F32, tag="cs_dr",
                                            name=f"csdr{tagp}{rep}_{mblk}")
                        nc.sync.dma_start(cs_dr, cs_sb)
                        rec = small.tile([P, 2], F32, tag="rec",
                                         name=f"rec{tagp}{rep}_{mblk}")
                        nc.sync.dma_start(
                            rec, cs_dr.rearrange("o (t p) -> (o p) t", p=P))
                        nc.vector.reciprocal(rec, rec)

                        for mt2 in range(2):
                            mt = 2 * mblk + mt2
                            res_t = resid_p.tile([P, D], F32, tag="res",
                                                 name=f"res{tagp}{rep}_{mt}")
                            nc.sync.dma_start(
                                res_t, resid_dram.ap()[mt * P:(mt + 1) * P, :])
                            raw = raw_p.tile([P, D], F32, tag="raw",
                                             name=f"raw{tagp}{rep}_{mt}")
                            for d_ in range(2):
                                nc.vector.scalar_tensor_tensor(
                                    raw[:, d_ * 512:(d_ + 1) * 512],
                                    o_ps[mt2][d_], rec[:, mt2:mt2 + 1],
                                    res_t[:, d_ * 512:(d_ + 1) * 512],
                                    ALU.mult, ALU.add)
                            xo = lnout_p.tile([P, D], F32, tag="lnout",
                                              name=f"xo{tagp}{rep}_{mt}")
                            _layernorm(nc, small, raw, xo, eps_t, gamma_t, beta_t)
                            nc.sync.dma_start(
                                spill_dram.ap()[mt * P:(mt + 1) * P, :], xo)
                            for dcol in range(NDC):
                                tp = psum.tile([P, 512], F32, tag="st", bufs=3,
                                               name=f"tp{tagp}{rep}_{mt}_{dcol}")
                                nc.tensor.transpose(
                                    tp[:, :P], xo[:, dcol * P:(dcol + 1) * P], ident)
                                nc.vector.tensor_copy(
                                    xT_out[:, dcol, mt * P:(mt + 1) * P],
                                    tp[:, :P])

                attention(kT_d, v1_d, qT_sb, True, yres_d,
                          x1_d if phases >= 2 else out_d,
                          g1_t, be1_t, x1T_sb, "s")
                if phases < 2:
                    return
                attention(zT_d, v2_d, x1T_sb, False, x1_d,
                          x2_d if phases >= 3 else out_d,
                          g2_t, be2_t, x2T_sb, "c")
                if phases < 3:
                    return

                # ---- FFN + final AddNorm (gamma3/beta3 applied on host) ----
                hT = persist.tile([P, NFT, FBLK], BF16, tag="hT", name=f"hT{rep}")
                for mb in range(NFB):
                    for fg in range(NFT // 2):
                        w1c = w1_p.tile([P, NDC, 2 * P], BF16, tag="w1c",
                                        name=f"w1c{rep}_{mb}_{fg}")
                        nc.sync.dma_start(
                            w1c,
                            w1_d.ap().rearrange("(c p) f -> p c f", p=P)
                            [:, :, fg * 2 * P:(fg + 1) * 2 * P])
                        for f2 in range(2):
                            ft = fg * 2 + f2
                            h_ps = psum.tile([P, 512], F32, tag="st", bufs=3,
                                             name=f"hps{rep}_{mb}_{ft}")
                            hv = h_ps[:, :FBLK]
                            for dc in range(NDC):
                                nc.tensor.matmul(
                                    hv,
                                    w1c[:, dc, f2 * P:(f2 + 1) * P],
                                    x2T_sb[:, dc, mb * FBLK:(mb + 1) * FBLK],
                                    start=(dc == 0), stop=(dc == NDC - 1))
                            nc.scalar.activation(hT[:, ft, :], hv, AF.Relu,
                                                 bias=b1c_t[:, ft:ft + 1])
                    ff_ps = [[psum.tile([P, 512], F32, tag="acc", bufs=4,
                                        name=f"ffps{rep}_{mb}_{t}_{d_}")
                              for d_ in range(2)] for t in range(2)]
                    for d_ in range(2):
                        for ft in range(NFT):
                            w2c = w2_p.tile([P, 512], BF16, tag="w2c",
                                            name=f"w2c{rep}_{mb}_{d_}_{ft}")
                            nc.sync.dma_start(
                                w2c,
                                w2_d.ap()[ft * P:(ft + 1) * P,
                                          d_ * 512:(d_ + 1) * 512])
                            for mt2 in range(2):
                                nc.tensor.matmul(
                                    ff_ps[mt2][d_],
                                    hT[:, ft, mt2 * P:(mt2 + 1) * P],
                                    w2c,
                                    start=(ft == 0), stop=(ft == NFT - 1))
                    for mt2 in range(2):
                        mt = 2 * mb + mt2
                        res_t = resid_p.tile([P, D], F32, tag="res",
                                             name=f"resf{rep}_{mt}")
                        nc.sync.dma_start(
                            res_t, x2_d.ap()[mt * P:(mt + 1) * P, :])
                        raw = raw_p.tile([P, D], F32, tag="raw",
                                         name=f"rawf{rep}_{mt}")
                        for d_ in range(2):
                            sl = slice(d_ * 512, (d_ + 1) * 512)
                            nc.vector.tensor_add(raw[:, sl], ff_ps[mt2][d_],
                                                 res_t[:, sl])
                            nc.vector.tensor_add(raw[:, sl], raw[:, sl],
                                                 b2_t[:, sl])
                        xo = lnout_p.tile([P, D], F32, tag="lnout",
                                          name=f"xof{rep}_{mt}")
                        _layernorm(nc, small, raw, xo, eps_t, None, None)
                        nc.sync.dma_start(out_d.ap()[mt * P:(mt + 1) * P, :], xo)

            if reps == 1:
                body(0)
            else:
                # hardware loop: same NEFF size, repeats the whole block so
                # wall-time deltas isolate per-iteration HW time
                with tc.For_i(0, reps, 1):
                    body(0)

    nc.compile()
    return nc


def _prep_core_inputs(y, Z, w1b, w2b, b1c, b2, g1b, be1b, g2b, be2b, b_idx, h):
    yb = y[b_idx]
    zb = Z[b_idx]
    yb_bf = yb.astype(bf)
    zb_bf = zb.astype(bf)
    kT = np.ascontiguousarray(yb_bf.T)
    qT = np.ascontiguousarray(kT[:, h::2])
    zT = np.ascontiguousarray(zb_bf.T)
    yres = np.ascontiguousarray(yb[h::2])

    # mask[p, kt, j]: key (kt*128+p) visible to query (2*(256*(kt//4)+j)+h)
    p_i = np.arange(P)[:, None, None]
    kt_i = np.arange(NKT)[None, :, None]
    j_i = np.arange(MBLK)[None, None, :]
    k_idx = kt_i * P + p_i
    q_idx = 2 * (MBLK * (kt_i // 4) + j_i) + h
    mask = (k_idx <= q_idx).astype(bf)

    return {
        "qT": qT, "kT": kT, "v1": yb_bf, "zT": zT, "v2": zb_bf,
        "yres": yres, "mask": np.ascontiguousarray(mask),
        "w1": w1b, "w2": w2b, "b1c": b1c, "b2v": b2,
        "g1v": g1b, "be1v": be1b, "g2v": g2b, "be2v": be2b,
    }


def make_in_maps(y, Z, w1, b1, w2, b2, g1, beta1, g2, beta2):
    w1b = w1.astype(bf)
    w2b = w2.astype(bf)
    b1c = np.ascontiguousarray(b1.reshape(NFT, P).T.astype(np.float32))
    args = (y, Z, w1b, w2b, b1c, b2.astype(np.float32),
            g1.astype(bf), beta1.astype(bf), g2.astype(bf), beta2.astype(bf))
    return [_prep_core_inputs(*args, c // 2, c % 2) for c in range(8)]


def kernel(y, Z, w1, b1, w2, b2, g1, beta1, g2, beta2, g3, beta3):
    y = np.asarray(y, dtype=np.float32)
    Z = np.asarray(Z, dtype=np.float32)
    (w1, b1, w2, b2, g1, beta1, g2, beta2, g3, beta3) = [
        np.asarray(a, dtype=np.float32)
        for a in (w1, b1, w2, b2, g1, beta1, g2, beta2, g3, beta3)]

    in_maps = make_in_maps(y, Z, w1, b1, w2, b2, g1, beta1, g2, beta2)
    nc = build_nc(1)
    res = run_bass_kernel_spmd(nc, in_maps, core_ids=list(range(8)), trace=False)

    out = np.empty((B, S, D), np.float32)
    for c in range(8):
        out[c // 2, c % 2::2, :] = res.results[c]["out"]
    # final gamma/beta exact in fp32 on host
    if not (np.all(g3 == 1.0) and np.all(beta3 == 0.0)):
        out = out * g3 + beta3
    return out


# revision 10
# speedup vs baseline: 12622.6017x; 2.2978x over previous
"""Trainium2 Bass kernel for a transformer decoder block (self-attn + cross-attn + FFN,
each with residual AddNorm), distributed over 8 NeuronCores.

Sharding: core c -> (batch b = c//2, row-interleave h = c%2). Each core owns the
1024 query rows y[b, h::2] of one batch element. All phases (attention outputs,
layernorms, FFN) are row-local, so no collectives are needed. Interleaving the
causal rows (global q = 2*m + h) makes the causal skip pattern identical on
every core, so one SPMD program can statically skip fully-masked key tiles.

Layout strategy (avoids all on-chip transposes in attention):
  scores^T St[k, m] = K·Qᵀ via lhsT=Kᵀ (d-major), rhs=Qᵀ (d-major)
  softmax denominator via ones-vector matmul (sum over key partitions)
  attn_out[m, d] = expStᵀ·V via lhsT=expSt, rhs=V (natural row-major)
FFN: hᵀ[f, m] = relu(w1ᵀ·x2ᵀ + b1) via lhsT=w1 (natural), rhs=x2ᵀ;
     ff[m, d] = hᵀᵀ·w2 via lhsT=hᵀ, rhs=w2 (natural).
Only x1 and x2 need on-chip transposes (PE identity transpose, 64 tiles each).

Matmuls run in bf16 (fp32 accumulation in PSUM); softmax/layernorm math in fp32.
gamma3/beta3 are applied on the host after gathering (exact fp32); the inner
gammas/betas and biases are applied on-device.
"""
import functools

import numpy as np
import ml_dtypes

import concourse.bacc as bacc
import concourse.bass as bass
import concourse.mybir as mybir
import concourse.tile as tile
from concourse.bass_utils import run_bass_kernel_spmd
from concourse.masks import make_identity

BF16 = mybir.dt.bfloat16
F32 = mybir.dt.float32
AF = mybir.ActivationFunctionType
ALU = mybir.AluOpType

P = 128
B, S, D, DFF = 4, 2048, 1024, 4096
M = S // 2              # local query rows per core
LK = S                  # key length
NDC = D // P            # 8 contraction chunks over d
NKT = LK // P           # 16 key tiles
MBLK = 256              # query-block size in the attention phases
NMBLK = M // MBLK       # 4
FBLK = 256              # m-block size in the FFN phase
NFB = M // FBLK         # 4
NFT = DFF // P          # 32 f tiles
EPS = 1e-5
SCALE = 1.0 / np.sqrt(D).item()

bf = ml_dtypes.bfloat16


def _bcast_ap(handle, n):
    """DRAM [n] vector -> partition-broadcast AP [P, n] (stride-0 partition dim)."""
    ap = handle.ap()
    return bass.AP(ap.tensor, ap.offset, [[0, P]] + list(ap.ap))


def _layernorm(nc, small, raw, out, eps_t, gamma_t, beta_t):
    """out = (raw - mean)/sqrt(var+eps) * gamma + beta, rows on partitions."""
    stats = small.tile([P, 2, 6], F32, tag="stats", name="stats")
    nc.vector.bn_stats(stats[:, 0, :], raw[:, 0:512])
    nc.vector.bn_stats(stats[:, 1, :], raw[:, 512:1024])
    mv = small.tile([P, 2], F32, tag="mv", name="mv")
    nc.vector.bn_aggr(mv, stats)
    rstd = small.tile([P, 1], F32, tag="rstd", name="rstd")
    nc.scalar.activation(rstd, mv[:, 1:2], AF.Sqrt, bias=eps_t)
    nc.vector.reciprocal(rstd, rstd)
    nc.vector.tensor_scalar(out, raw, mv[:, 0:1], rstd, ALU.subtract, ALU.mult)
    if gamma_t is not None:
        nc.vector.tensor_mul(out, out, gamma_t)
    if beta_t is not None:
        nc.vector.tensor_add(out, out, beta_t)


import os
_NMBLK_LIM = int(os.environ.get("K_NMBLK", str(NMBLK)))


@functools.lru_cache(maxsize=2)
def build_nc(reps: int = 1, phases: int = 3):
    nc = bacc.Bacc("TRN2", target_bir_lowering=False, debug=False)

    # ---- I/O ----
    qT_d = nc.dram_tensor("qT", [D, M], BF16, kind="ExternalInput")
    kT_d = nc.dram_tensor("kT", [D, LK], BF16, kind="ExternalInput")
    v1_d = nc.dram_tensor("v1", [LK, D], BF16, kind="ExternalInput")
    zT_d = nc.dram_tensor("zT", [D, LK], BF16, kind="ExternalInput")
    v2_d = nc.dram_tensor("v2", [LK, D], BF16, kind="ExternalInput")
    yres_d = nc.dram_tensor("yres", [M, D], F32, kind="ExternalInput")
    mask_d = nc.dram_tensor("mask", [P, NKT, MBLK], BF16, kind="ExternalInput")
    w1_d = nc.dram_tensor("w1", [D, DFF], BF16, kind="ExternalInput")
    w2_d = nc.dram_tensor("w2", [DFF, D], BF16, kind="ExternalInput")
    b1c_d = nc.dram_tensor("b1c", [P, NFT], F32, kind="ExternalInput")
    b2_d = nc.dram_tensor("b2v", [D], F32, kind="ExternalInput")
    g1_d = nc.dram_tensor("g1v", [D], BF16, kind="ExternalInput")
    be1_d = nc.dram_tensor("be1v", [D], BF16, kind="ExternalInput")
    g2_d = nc.dram_tensor("g2v", [D], BF16, kind="ExternalInput")
    be2_d = nc.dram_tensor("be2v", [D], BF16, kind="ExternalInput")
    out_d = nc.dram_tensor("out", [M, D], F32, kind="ExternalOutput")

    x1_d = nc.dram_tensor("x1_scratch", [M, D], F32)
    x2_d = nc.dram_tensor("x2_scratch", [M, D], F32)

    with tile.TileContext(nc) as tc:
        with (
            tc.tile_pool(name="const", bufs=1) as const,
            tc.tile_pool(name="persist", bufs=1) as persist,
            tc.tile_pool(name="est_p", bufs=3) as est_p,
            tc.tile_pool(name="resid_p", bufs=2) as resid_p,
            tc.tile_pool(name="raw_p", bufs=4) as raw_p,
            tc.tile_pool(name="lnout_p", bufs=4) as lnout_p,
            tc.tile_pool(name="w1_p", bufs=2) as w1_p,
            tc.tile_pool(name="w2_p", bufs=3) as w2_p,
            tc.tile_pool(name="small", bufs=4) as small,
            tc.tile_pool(name="dramsc", bufs=2, space="DRAM") as dramsc,
            tc.tile_pool(name="psum", bufs=1, space="PSUM") as psum,
        ):
            # ---- constants ----
            ones_t = const.tile([P, 1], BF16, name="ones_t")
            nc.vector.memset(ones_t, 1.0)
            eps_t = const.tile([P, 1], F32, name="eps_t")
            nc.vector.memset(eps_t, EPS)
            ident = const.tile([P, P], F32, name="ident")
            make_identity(nc, ident)
            b1c_t = const.tile([P, NFT], F32, name="b1c_t")
            nc.sync.dma_start(b1c_t, b1c_d.ap())
            b2_t = const.tile([P, D], F32, name="b2_t")
            nc.sync.dma_start(b2_t, _bcast_ap(b2_d, D))
            g1_t = const.tile([P, D], BF16, name="g1_t")
            nc.sync.dma_start(g1_t, _bcast_ap(g1_d, D))
            be1_t = const.tile([P, D], BF16, name="be1_t")
            nc.sync.dma_start(be1_t, _bcast_ap(be1_d, D))
            g2_t = const.tile([P, D], BF16, name="g2_t")
            nc.sync.dma_start(g2_t, _bcast_ap(g2_d, D))
            be2_t = const.tile([P, D], BF16, name="be2_t")
            nc.sync.dma_start(be2_t, _bcast_ap(be2_d, D))
            mask_t = const.tile([P, NKT, MBLK], BF16, name="mask_t")
            nc.sync.dma_start(mask_t, mask_d.ap())

            def body(rep):
                # persistent per-phase tensors (tags shared across phases where
                # the successor is DMA-loaded, so reuse only gates a DMA)
                qT_sb = persist.tile([P, NDC, M], BF16, tag="qT", name=f"qT_sb{rep}")
                nc.sync.dma_start(qT_sb, qT_d.ap().rearrange("(c p) m -> p c m", p=P))
                x1T_sb = persist.tile([P, NDC, M], BF16, tag="x1T", name=f"x1T{rep}")
                x2T_sb = persist.tile([P, NDC, M], BF16, tag="x2T", name=f"x2T{rep}")

                def attention(kvT_d_, v_d_, q_sb, causal, resid_dram, spill_dram,
                              gamma_t, beta_t, xT_out, tagp):
                    kvT = persist.tile([P, NDC, LK], BF16, tag="kvT",
                                       name=f"kvT_{tagp}{rep}")
                    nc.sync.dma_start(
                        kvT, kvT_d_.ap().rearrange("(c p) k -> p c k", p=P))
                    v_sb = persist.tile([P, NKT, D], BF16, tag="V",
                                        name=f"v_{tagp}{rep}")
                    nc.sync.dma_start(
                        v_sb, v_d_.ap().rearrange("(c p) d -> p c d", p=P))

                    # deferred PE transposes (emitted one mblk late so the PE
                    # never waits in-line on the LN chain that produces xo)
                    pending_tp = []

                    def flush_tp():
                        while pending_tp:
                            xo, mt = pending_tp.pop(0)
                            for dcol in range(NDC):
                                tp = psum.tile([P, 512], F32, tag="st", bufs=3,
                                               name=f"tp{tagp}{rep}_{mt}_{dcol}")
                                nc.tensor.transpose(
                                    tp[:, :P], xo[:, dcol * P:(dcol + 1) * P],
                                    ident)
                                nc.vector.tensor_copy(
                                    xT_out[:, dcol, mt * P:(mt + 1) * P],
                                    tp[:, :P])

                    def st_group(mblk, kt):
                        """St matmuls + exp (+ causal mask) for one key tile."""
                        st_ps = psum.tile([P, 512], F32, tag="st", bufs=3,
                                          name=f"st{tagp}{rep}_{mblk}_{kt}")
                        stv = st_ps[:, :MBLK]
                        for dc in range(NDC):
                            nc.tensor.matmul(
                                stv,
                                kvT[:, dc, kt * P:(kt + 1) * P],
                                q_sb[:, dc, mblk * MBLK:(mblk + 1) * MBLK],
                                start=(dc == 0), stop=(dc == NDC - 1))
                        est = est_p.tile([P, MBLK], BF16, tag="est",
                                         name=f"est{tagp}{rep}_{mblk}_{kt}")
                        nc.scalar.activation(est, stv, AF.Exp, scale=SCALE)
                        if causal and kt >= 4 * mblk:
                            nc.vector.tensor_mul(est, est, mask_t[:, kt, :])
                        return est

                    for mblk in range(_NMBLK_LIM):
                        nkt = 4 * mblk + 4 if causal else NKT
                        o_ps = [[psum.tile([P, 512], F32, tag="acc", bufs=4,
                                           name=f"ops{tagp}{rep}_{mblk}_{t}_{d_}")
                                 for d_ in range(2)] for t in range(2)]
                        cs_ps = psum.tile([1, MBLK], F32, tag="csum", bufs=1,
                                          name=f"cs{tagp}{rep}_{mblk}")
                        # software pipeline: St(kt+1) issues before the PE
                        # consumes est(kt), hiding the ACT exp latency
                        est_next = st_group(mblk, 0)
                        for kt in range(nkt):
                            est = est_next
                            if kt + 1 < nkt:
                                est_next = st_group(mblk, kt + 1)
                            elif pending_tp:
                                flush_tp()
                            nc.tensor.matmul(cs_ps, ones_t, est,
                                             start=(kt == 0), stop=(kt == nkt - 1))
                            for mt2 in range(2):
                                for d_ in range(2):
                                    nc.tensor.matmul(
                                        o_ps[mt2][d_],
                                        est[:, mt2 * P:(mt2 + 1) * P],
                                        v_sb[:, kt, d_ * 512:(d_ + 1) * 512],
                                        start=(kt == 0), stop=(kt == nkt - 1))
                        # denominators: psum row -> dram bounce -> [P, 2] -> 1/x
                        cs_sb = small.tile([1, MBLK], F32, tag="cs_sb",
                                           name=f"cssb{tagp}{rep}_{mblk}")
                        nc.vector.tensor_copy(cs_sb, cs_ps)
                        cs_dr = dramsc.tile([1, MBLK], F32, tag="cs_dr",
                                            name=f"csdr{tagp}{rep}_{mblk}")
                        nc.sync.dma_start(cs_dr, cs_sb)
                        rec = small.tile([P, 2], F32, tag="rec",
                                         name=f"rec{tagp}{rep}_{mblk}")
                        nc.sync.dma_start(
                            rec, cs_dr.rearrange("o (t p) -> (o p) t", p=P))
                        nc.vector.reciprocal(rec, rec)

                        for mt2 in range(2):
                            mt = 2 * mblk + mt2
                            res_t = resid_p.tile([P, D], F32, tag="res",
                                                 name=f"res{tagp}{rep}_{mt}")
                            nc.sync.dma_start(
                                res_t, resid_dram.ap()[mt * P:(mt + 1) * P, :])
                            raw = raw_p.tile([P, D], F32, tag="raw",
                                             name=f"raw{tagp}{rep}_{mt}")
                            for d_ in range(2):
                                nc.vector.scalar_tensor_tensor(
                                    raw[:, d_ * 512:(d_ + 1) * 512],
                                    o_ps[mt2][d_], rec[:, mt2:mt2 + 1],
                                    res_t[:, d_ * 512:(d_ + 1) * 512],
                                    ALU.mult, ALU.add)
                            xo = lnout_p.tile([P, D], F32, tag="lnout",
                                              name=f"xo{tagp}{rep}_{mt}")
                            _layernorm(nc, small, raw, xo, eps_t, gamma_t, beta_t)
                            nc.sync.dma_start(
                                spill_dram.ap()[mt * P:(mt + 1) * P, :], xo)
                            pending_tp.append((xo, mt))
                    flush_tp()

                attention(kT_d, v1_d, qT_sb, True, yres_d,
                          x1_d if phases >= 2 else out_d,
                          g1_t, be1_t, x1T_sb, "s")
                if phases < 2:
                    return
                attention(zT_d, v2_d, x1T_sb, False, x1_d,
                          x2_d if phases >= 3 else out_d,
                          g2_t, be2_t, x2T_sb, "c")
                if phases < 3:
                    return

                # ---- FFN + final AddNorm (gamma3/beta3 applied on host) ----
                hT = persist.tile([P, NFT, FBLK], BF16, tag="hT", name=f"hT{rep}")
                for mb in range(NFB):
                    for fg in range(NFT // 2):
                        w1c = w1_p.tile([P, NDC, 2 * P], BF16, tag="w1c",
                                        name=f"w1c{rep}_{mb}_{fg}")
                        nc.sync.dma_start(
                            w1c,
                            w1_d.ap().rearrange("(c p) f -> p c f", p=P)
                            [:, :, fg * 2 * P:(fg + 1) * 2 * P])
                        for f2 in range(2):
                            ft = fg * 2 + f2
                            h_ps = psum.tile([P, 512], F32, tag="st", bufs=3,
                                             name=f"hps{rep}_{mb}_{ft}")
                            hv = h_ps[:, :FBLK]
                            for dc in range(NDC):
                                nc.tensor.matmul(
                                    hv,
                                    w1c[:, dc, f2 * P:(f2 + 1) * P],
                                    x2T_sb[:, dc, mb * FBLK:(mb + 1) * FBLK],
                                    start=(dc == 0), stop=(dc == NDC - 1))
                            nc.scalar.activation(hT[:, ft, :], hv, AF.Relu,
                                                 bias=b1c_t[:, ft:ft + 1])
                    ff_ps = [[psum.tile([P, 512], F32, tag="acc", bufs=4,
                                        name=f"ffps{rep}_{mb}_{t}_{d_}")
                              for d_ in range(2)] for t in range(2)]
                    for d_ in range(2):
                        for ft in range(NFT):
                            w2c = w2_p.tile([P, 512], BF16, tag="w2c",
                                            name=f"w2c{rep}_{mb}_{d_}_{ft}")
                            nc.sync.dma_start(
                                w2c,
                                w2_d.ap()[ft * P:(ft + 1) * P,
                                          d_ * 512:(d_ + 1) * 512])
                            for mt2 in range(2):
                                nc.tensor.matmul(
                                    ff_ps[mt2][d_],
                                    hT[:, ft, mt2 * P:(mt2 + 1) * P],
                                    w2c,
                                    start=(ft == 0), stop=(ft == NFT - 1))
                    for mt2 in range(2):
                        mt = 2 * mb + mt2
                        res_t = resid_p.tile([P, D], F32, tag="res",
                                             name=f"resf{rep}_{mt}")
                        nc.sync.dma_start(
                            res_t, x2_d.ap()[mt * P:(mt + 1) * P, :])
                        raw = raw_p.tile([P, D], F32, tag="raw",
                                         name=f"rawf{rep}_{mt}")
                        for d_ in range(2):
                            sl = slice(d_ * 512, (d_ + 1) * 512)
                            nc.vector.tensor_add(raw[:, sl], ff_ps[mt2][d_],
                                                 res_t[:, sl])
                            nc.vector.tensor_add(raw[:, sl], raw[:, sl],
                                                 b2_t[:, sl])
                        xo = lnout_p.tile([P, D], F32, tag="lnout",
                                          name=f"xof{rep}_{mt}")
                        _layernorm(nc, small, raw, xo, eps_t, None, None)
                        nc.sync.dma_start(out_d.ap()[mt * P:(mt + 1) * P, :], xo)

            if reps == 1:
                body(0)
            else:
                # hardware loop: same NEFF size, repeats the whole block so
                # wall-time deltas isolate per-iteration HW time
                with tc.For_i(0, reps, 1):
                    body(0)

    nc.compile()
    return nc


def _prep_core_inputs(y, Z, w1b, w2b, b1c, b2, g1b, be1b, g2b, be2b, b_idx, h):
    yb = y[b_idx]
    zb = Z[b_idx]
    yb_bf = yb.astype(bf)
    zb_bf = zb.astype(bf)
    kT = np.ascontiguousarray(yb_bf.T)
    qT = np.ascontiguousarray(kT[:, h::2])
    zT = np.ascontiguousarray(zb_bf.T)
    yres = np.ascontiguousarray(yb[h::2])

    # mask[p, kt, j]: key (kt*128+p) visible to query (2*(256*(kt//4)+j)+h)
    p_i = np.arange(P)[:, None, None]
    kt_i = np.arange(NKT)[None, :, None]
    j_i = np.arange(MBLK)[None, None, :]
    k_idx = kt_i * P + p_i
    q_idx = 2 * (MBLK * (kt_i // 4) + j_i) + h
    mask = (k_idx <= q_idx).astype(bf)

    return {
        "qT": qT, "kT": kT, "v1": yb_bf, "zT": zT, "v2": zb_bf,
        "yres": yres, "mask": np.ascontiguousarray(mask),
        "w1": w1b, "w2": w2b, "b1c": b1c, "b2v": b2,
        "g1v": g1b, "be1v": be1b, "g2v": g2b, "be2v": be2b,
    }


def make_in_maps(y, Z, w1, b1, w2, b2, g1, beta1, g2, beta2):
    w1b = w1.astype(bf)
    w2b = w2.astype(bf)
    b1c = np.ascontiguousarray(b1.reshape(NFT, P).T.astype(np.float32))
    args = (y, Z, w1b, w2b, b1c, b2.astype(np.float32),
            g1.astype(bf), beta1.astype(bf), g2.astype(bf), beta2.astype(bf))
    return [_prep_core_inputs(*args, c // 2, c % 2) for c in range(8)]


def kernel(y, Z, w1, b1, w2, b2, g1, beta1, g2, beta2, g3, beta3):
    y = np.asarray(y, dtype=np.float32)
    Z = np.asarray(Z, dtype=np.float32)
    (w1, b1, w2, b2, g1, beta1, g2, beta2, g3, beta3) = [
        np.asarray(a, dtype=np.float32)
        for a in (w1, b1, w2, b2, g1, beta1, g2, beta2, g3, beta3)]

    in_maps = make_in_maps(y, Z, w1, b1, w2, b2, g1, beta1, g2, beta2)
    nc = build_nc(1)
    res = run_bass_kernel_spmd(nc, in_maps, core_ids=list(range(8)), trace=False)

    out = np.empty((B, S, D), np.float32)
    for c in range(8):
        out[c // 2, c % 2::2, :] = res.results[c]["out"]
    # final gamma/beta exact in fp32 on host
    if not (np.all(g3 == 1.0) and np.all(beta3 == 0.0)):
        out = out * g3 + beta3
    return out
